# revision 15
# baseline (speedup 1.0000x reference)
"""Fused attention block (QKV conv -> 4-head attention -> proj -> BatchNorm -> LeakyReLU)
distributed over 8 trn2 NeuronCores, data-parallel over the batch dim.

Self-contained: hardcodes shapes B=8, C=64, N=2048, H=4.

The kernel is exp-throughput bound (H*N*N = 16.8M exps/core), so the design
keeps both exp-capable engines (ScalarE ACT, VectorE DVE) saturated and hides
everything else under them:
  - scores computed transposed (S^T = K^T Q, keys on partitions) in
    [128 x 512] PSUM tiles through a 5-slot ring; the 4 heads' score matmuls
    issue back-to-back into distinct PE row-groups -> 4x tile concurrency;
  - exp split by head with per-softmax-row purity: heads 0,2 exact exp on
    ScalarE; heads 1,3 on VectorE via a bf16-Schraudolph bit trick
    (int16(FA*s + FB) reinterpreted as bf16 ~= exp(s*scale)); the ~3%
    sawtooth is common to numerator and denominator of those rows and
    largely cancels;
  - PV matmuls col-group tiled (4x concurrent), softmax denominators come
    free from a ones-column folded into the stationary V^T operand;
  - per-query normalization deferred: 1/denom = exp(-ln d) on ScalarE in a
    compact [128,32] layout (DRAM bounce for the transpose), then one
    broadcast-multiply;
  - BatchNorm stats all-reduced across cores ([128,4] f32), folded+replicated
    via a DRAM bounce, inv_std = exp(-0.5*ln(var+eps)); cheap SBUF-only
    elementwise tail ops run on the otherwise idle GpSimd.
"""
import numpy as np
import ml_dtypes

import concourse.bass as bass
import concourse.mybir as mybir
from concourse import bacc, tile
from concourse.bass_utils import run_bass_kernel_spmd

B, C, N, H, D = 8, 64, 2048, 4, 16
C2 = 2 * C           # 128 input channels after concat
NC = 1024            # query-dim chunk
NCH = N // NC        # 2 chunks
MT = N // 128        # 16 key tiles of 128
F32 = mybir.dt.float32
BF16 = mybir.dt.bfloat16
I16 = mybir.dt.int16
SCALE = float(D) ** -0.5
BN_EPS = 1e-5
LEAK = 0.2
N_CORES = 8
CNT = float(B * N)   # batchnorm population count

# bf16-Schraudolph: bf16 bits of exp(s*SCALE) ~= int16(FA*s + FB)
_LN2 = float(np.log(2.0))
FA = 128.0 / _LN2 * SCALE
FB = 127.0 * 128.0 - 4.75

ACT_HEADS = (0, 2)   # exact exp on ScalarE
                     # heads 1,3: fast exp on VectorE

Alu = mybir.AluOpType
Act = mybir.ActivationFunctionType


def build():
    nc = bacc.Bacc("TRN2", target_bir_lowering=False, debug=False,
                   num_devices=N_CORES)
    x_p = nc.declare_dram_parameter("x", [C2, N], BF16, isOutput=False)
    wq_p = nc.declare_dram_parameter("wq", [C2, 128], BF16, isOutput=False)
    wk_p = nc.declare_dram_parameter("wk", [C2, 128], BF16, isOutput=False)
    wv_p = nc.declare_dram_parameter("wv", [C2, C], BF16, isOutput=False)
    wp_p = nc.declare_dram_parameter("wp", [C2, C], BF16, isOutput=False)
    g_p = nc.declare_dram_parameter("gamma", [C, 1], F32, isOutput=False)
    b_p = nc.declare_dram_parameter("beta", [C, 1], F32, isOutput=False)
    out_p = nc.declare_dram_parameter("out", [C, N], F32, isOutput=True)

    with tile.TileContext(nc) as tc:
        with (
            tc.tile_pool(name="sb", bufs=1) as sb,
            tc.tile_pool(name="ps_a", bufs=3, space="PSUM") as ps_a,
            tc.tile_pool(name="ps_d", bufs=3, space="PSUM") as ps_d,
            tc.tile_pool(name="ps_pv", bufs=1, space="PSUM") as ps_pv,
            tc.tile_pool(name="pp", bufs=4) as pp,
            tc.tile_pool(name="fp", bufs=4) as fp,
            tc.tile_pool(name="ep", bufs=2) as ep,
            tc.tile_pool(name="dram", bufs=2, space="DRAM") as dram,
        ):
            # ---- persistent SBUF tiles
            x_sb = sb.tile([C2, N], BF16, tag="x")
            wq_sb = sb.tile([C2, 128], BF16, tag="wq")
            wk_sb = sb.tile([C2, 128], BF16, tag="wk")
            wv_sb = sb.tile([C2, C], BF16, tag="wv")
            wp_sb = sb.tile([C2, C], BF16, tag="wp")
            g_sb = sb.tile([C2, 1], F32, tag="g")     # gamma replicated x2
            b_sb = sb.tile([C2, 1], F32, tag="b")
            q_sb = sb.tile([C2, N], BF16, tag="q")    # head h rows 32h..32h+16
            k_sb = sb.tile([C2, N], BF16, tag="k")
            # per key-tile, per head: 32 cols = [16 V^T | 1 ones | 15 zeros]
            vt_sb = sb.tile([C2, MT * 128], BF16, tag="vt")
            y_sb = sb.tile([C2, NC], F32, tag="y")    # proj out, fold layout
            stats = sb.tile([C2, 4], F32, tag="stats")  # sum_j0 sum_j1 sq_j0 sq_j1

            # spread prologue loads across engines' DMA queues
            nc.sync.dma_start(x_sb[:, 0:NC], x_p[:, 0:NC])
            nc.gpsimd.dma_start(wq_sb[:], wq_p[:])
            nc.gpsimd.dma_start(wk_sb[:], wk_p[:])
            nc.scalar.dma_start(x_sb[:, NC:N], x_p[:, NC:N])
            nc.sync.dma_start(wv_sb[:], wv_p[:])
            nc.sync.dma_start(wp_sb[:], wp_p[:])
            nc.gpsimd.dma_start(g_sb[0:C, :], g_p[:])
            nc.gpsimd.dma_start(g_sb[C:C2, :], g_p[:])
            nc.scalar.dma_start(b_sb[0:C, :], b_p[:])
            nc.scalar.dma_start(b_sb[C:C2, :], b_p[:])

            # ---- QKV projections (heads padded to 32-partition groups).
            # Evacuate per 512-col piece, alternating DVE/ACT so the first
            # score matmul isn't gated on one engine.
            def qk_chunk(dst, w, c, eng_flip):
                for j in range(NC // 512):
                    qp = ps_a.tile([C2, 512], F32, tag="sa", name="qp")
                    nc.tensor.matmul(
                        qp[:], lhsT=w[:],
                        rhs=x_sb[:, NC * c + 512 * j: NC * c + 512 * (j + 1)])
                    dslice = dst[:, NC * c + 512 * j: NC * c + 512 * (j + 1)]
                    if (j + eng_flip) % 2 == 0:
                        nc.vector.tensor_copy(dslice, qp[:])
                    else:
                        nc.scalar.copy(dslice, qp[:])

            qk_chunk(q_sb, wq_sb, 0, 0)
            qk_chunk(k_sb, wk_sb, 0, 1)
            qk_chunk(q_sb, wq_sb, 1, 0)
            qk_chunk(k_sb, wk_sb, 1, 1)

            # V^T zero fill + ones columns on the otherwise-idle gpsimd
            nc.gpsimd.memset(vt_sb[:], 0.0)
            ones_ap = vt_sb[:].rearrange(
                "q (p h e) -> q p h e", p=MT, h=H, e=32)[:, :, :, 16:17]
            nc.gpsimd.memset(ones_ap, 1.0)

            # V^T: 16 key tiles, two 512-col psum ring slots of 8 tiles each
            for half in range(2):
                vp = ps_a.tile([C2, 512], F32, tag="sa", name="vp")
                for p in range(8):
                    pk = 8 * half + p
                    nc.tensor.matmul(vp[:, C * p:C * (p + 1)],
                                     lhsT=x_sb[:, 128 * pk:128 * (pk + 1)],
                                     rhs=wv_sb[:])
                vt_dst = vt_sb[:, 1024 * half:1024 * (half + 1)].rearrange(
                    "q (p h e) -> q p h e", p=8, h=H, e=32)[:, :, :, 0:16]
                vt_src = vp[:].rearrange(
                    "q (p h d) -> q p h d", p=8, h=H, d=D)
                nc.vector.tensor_copy(vt_dst, vt_src)

            def epilogue_head(c, pv):
                """Evacuate pv to SBUF at the chunk boundary (frees the psum
                slot); split across ACT/DVE so neither queue stalls long."""
                pvs = ep.tile([C2, NC], F32, tag="pvs")
                nc.scalar.copy(pvs[:, 0:512], pv[:, 0:512])
                nc.vector.tensor_copy(pvs[:, 512:NC], pv[:, 512:NC])
                return pvs

            def epilogue_tail(c, pvs):
                """Normalize chunk-c attention output, project, evac + stats.
                Emitted a few steps into the NEXT chunk so the exp engines
                keep streaming; latency lives on DMA/GpSimd/PE instead."""
                # denominators pvs[32h+16, :] -> DRAM [4, NC] -> compact
                # [128, 32]; reciprocal on DVE (tiny); back out + bcast
                den_d = dram.tile([H, NC], F32, tag="den_d")
                for h in range(H):
                    nc.sync.dma_start(den_d[h:h + 1, :],
                                      pvs[32 * h + 16:32 * h + 17, :])
                den_c = ep.tile([C2, NC // 32], F32, tag="den_c")
                nc.sync.dma_start(
                    den_c[:],
                    den_d[:].rearrange("h (p q) -> (h p) q", p=32, q=32))
                rcp = ep.tile([C2, NC // 32], F32, tag="rcp")
                nc.vector.reciprocal(rcp[:], den_c[:])
                rec_d = dram.tile([H, NC], F32, tag="rec_d")
                nc.sync.dma_start(
                    rec_d[:].rearrange("h (p q) -> (h p) q", p=32, q=32),
                    rcp[:])
                rbc = ep.tile([C2, NC], F32, tag="rbc")
                for h in range(H):
                    nc.sync.dma_start(
                        rbc[32 * h:32 * h + 32, :],
                        rec_d[h:h + 1, :].partition_broadcast(32))
                on = ep.tile([C2, NC], BF16, tag="on")
                nc.gpsimd.tensor_tensor(on[:], pvs[:], rbc[:], op=Alu.mult)
                # projection into s-ring slots; chunk c rows 64c..64c+64
                r = slice(64 * c, 64 * c + 64)
                for j in range(NC // 512):
                    yp = ps_a.tile([C2, 512], F32, tag="sa", name="yp")
                    nc.tensor.matmul(yp[r, :],
                                     lhsT=wp_sb[:], rhs=on[:, 512 * j:512 * (j + 1)],
                                     tile_position=(0, 64 * c))
                    nc.vector.tensor_scalar(y_sb[r, 512 * j:512 * (j + 1)],
                                            yp[r, :], 1.0, 0.0,
                                            op0=Alu.mult, op1=Alu.add,
                                            accum_out=stats[r, j:j + 1])
                    ysq = ep.tile([C2, 512], F32, tag="ysq")
                    nc.vector.scalar_tensor_tensor(
                        ysq[r, :], y_sb[r, 512 * j:512 * (j + 1)], 0.0,
                        y_sb[r, 512 * j:512 * (j + 1)], op0=Alu.add,
                        op1=Alu.mult, accum_out=stats[r, 2 + j:3 + j])

            # ---- attention: per (chunk, key-tile, j-half): 4 S matmuls into
            # the 5-deep psum ring (distinct PE row-groups -> concurrent),
            # exp on ScalarE (heads 0,2) / VectorE (heads 1,3 bit-trick),
            # then 4 col-tiled PV matmuls accumulate into the pv slot.  The
            # PV group lags one (t,j) step so the PE emits the NEXT step's
            # score matmuls before the PV that waits on this step's exps —
            # the exp engines then never wait on a fresh score tile.
            pend_tail = None
            for c in range(NCH):
                pv = ps_pv.tile([C2, NC], F32, tag="pv", name="pv")
                pend = []

                def flush_pv(pv=pv):
                    for pt, t, j, h in pend.pop(0):
                        nc.tensor.matmul(
                            pv[32 * h:32 * h + 32, 512 * j:512 * (j + 1)],
                            lhsT=vt_sb[:, 128 * t + 32 * h: 128 * t + 32 * h + 32],
                            rhs=pt[:],
                            start=(t == 0), stop=(t == MT - 1),
                            skip_group_check=True,
                            tile_position=(0, 32 * h))

                for t in range(MT):
                    for j in range(NC // 512):
                        if c == 1 and t == 2 and j == 0 and pend_tail is not None:
                            epilogue_tail(*pend_tail)
                            pend_tail = None
                        sps = []
                        for h in range(H):
                            if h in ACT_HEADS:
                                sp = ps_a.tile([C2, 512], F32, tag="sa",
                                               name="spa")
                            else:
                                sp = ps_d.tile([C2, 512], F32, tag="sd",
                                               name="spd")
                            nc.tensor.matmul(
                                sp[:],
                                lhsT=k_sb[32 * h:32 * h + 16, 128 * t:128 * (t + 1)],
                                rhs=q_sb[32 * h:32 * h + 16,
                                         NC * c + 512 * j: NC * c + 512 * (j + 1)],
                                tile_position=(32 * h, 0))
                            sps.append((sp, h))
                        pts = []
                        for sp, h in sps:
                            if h in ACT_HEADS:
                                pt = pp.tile([C2, 512], BF16, tag="p")
                                nc.scalar.activation(pt[:], sp[:], Act.Exp,
                                                     scale=SCALE)
                                pts.append((pt[:], t, j, h))
                            else:
                                pti = fp.tile([C2, 512], I16, tag="pf")
                                nc.vector.tensor_scalar(pti[:], sp[:], FA, FB,
                                                        op0=Alu.mult,
                                                        op1=Alu.add)
                                pts.append((pti[:].bitcast(BF16), t, j, h))
                        pend.append(pts)
                        # flush with a 2-step lag: the PE then always has the
                        # next 2 steps' score matmuls dispatched ahead of the
                        # exp-gated PV group, so the exp engines never starve
                        if len(pend) > 2:
                            flush_pv()
                while pend:
                    flush_pv()
                pend_tail = (c, epilogue_head(c, pv))
            epilogue_tail(*pend_tail)

            # ---- cross-core reduce of batchnorm stats (prefold j-pairs
            # to [128, 2] so the collective moves 1KB)
            st2 = sb.tile([C2, 2], F32, tag="st2")
            nc.gpsimd.tensor_add(st2[:], stats[:, 0:4:2], stats[:, 1:4:2])
            st_in = dram.tile([C2, 2], F32, tag="st_in")
            st_out = dram.tile([N_CORES * C2, 2], F32, tag="st_out")
            nc.gpsimd.dma_start(st_in[:], st2[:])
            nc.gpsimd.collective_compute(
                "AllGather", Alu.bypass,
                replica_groups=[list(range(N_CORES))],
                ins=[st_in.opt()], outs=[st_out.opt()])
            # load all 8 ranks' [C2,2] as [C2, 16], reduce ranks, then fold
            # the two channel halves (replicated to all 128 partitions)
            ag = sb.tile([C2, 16], F32, tag="ag")
            nc.sync.dma_start(
                ag[:].rearrange("p (r j) -> p r j", r=N_CORES, j=2),
                st_out[:].rearrange("(r p) j -> p r j", r=N_CORES, p=C2))
            r8 = sb.tile([C2, 8], F32, tag="r8")
            nc.vector.tensor_add(r8[:], ag[:, 0:8], ag[:, 8:16])
            r4 = sb.tile([C2, 4], F32, tag="r4")
            nc.vector.tensor_add(r4[:], r8[:, 0:4], r8[:, 4:8])
            rsum = sb.tile([C2, 2], F32, tag="rsum")
            nc.vector.tensor_add(rsum[:], r4[:, 0:2], r4[:, 2:4])
            rs_d = dram.tile([C2, 2], F32, tag="rs_d")
            nc.sync.dma_start(rs_d[:], rsum[:])
            fa = sb.tile([C2, 2], F32, tag="fa")
            fb = sb.tile([C2, 2], F32, tag="fb")
            nc.sync.dma_start(fa[:], rs_d[:])
            nc.sync.dma_start(fb[0:C, :], rs_d[C:C2, :])
            nc.sync.dma_start(fb[C:C2, :], rs_d[0:C, :])
            # ---- finalize, single-engine chain on vector (ln/exp on ScalarE)
            me = sb.tile([C2, 2], F32, tag="me")     # [mean | E x^2]
            nc.vector.tensor_add(me[:], fa[:], fb[:])
            nc.vector.tensor_scalar_mul(me[:], me[:], 1.0 / CNT)
            mean = me[:, 0:1]
            msq = sb.tile([C2, 1], F32, tag="msq")
            nc.vector.tensor_mul(msq[:], me[:, 0:1], me[:, 0:1])
            var = sb.tile([C2, 1], F32, tag="var")
            nc.vector.tensor_sub(var[:], me[:, 1:2], msq[:])
            eps_t = sb.tile([C2, 1], F32, tag="eps")
            nc.gpsimd.memset(eps_t[:], BN_EPS)
            lnv = sb.tile([C2, 1], F32, tag="lnv")
            nc.scalar.activation(lnv[:], var[:], Act.Ln, bias=eps_t[:, 0:1])
            istd = sb.tile([C2, 1], F32, tag="istd")
            nc.scalar.activation(istd[:], lnv[:], Act.Exp, scale=-0.5)
            sc = sb.tile([C2, 1], F32, tag="sc")
            nc.vector.tensor_mul(sc[:], g_sb[:], istd[:])
            msc = sb.tile([C2, 1], F32, tag="msc")
            nc.vector.tensor_scalar(msc[:], mean, sc[:, 0:1], None, op0=Alu.mult)
            sh = sb.tile([C2, 1], F32, tag="sh")
            nc.vector.tensor_sub(sh[:], b_sb[:], msc[:])

            # ---- normalize + LeakyReLU + store (gpsimd + vector split)
            yn = ep.tile([C2, NC], F32, tag="rbc")
            nc.vector.tensor_scalar(yn[:], y_sb[:],
                                    sc[:, 0:1], sh[:, 0:1],
                                    op0=Alu.mult, op1=Alu.add)
            yl = ep.tile([C2, NC], F32, tag="on2")
            nc.vector.scalar_tensor_tensor(yl[:], yn[:], LEAK, yn[:],
                                           op0=Alu.mult, op1=Alu.max)

            nc.sync.dma_start(out_p[:, 0:NC], yl[0:C, :])
            nc.sync.dma_start(out_p[:, NC:N], yl[C:C2, :])

    nc.compile()

    # Post-compile surgery: one activation table set covers Exp+Ln; point the
    # first inserted load at it and drop the rest (loads are inserted after
    # semaphore generation, so they carry no sync state and removal is safe).
    from concourse.hw_specs import get_activation_tables
    tabs = list(get_activation_tables(nc.m.arch).keys())
    nle = tabs.index("natural_log_exp_and_others")
    loads = [(b, i) for b in nc.main_func.blocks for i in b.instructions
             if isinstance(i, mybir.InstLoadActFuncSet)]
    if loads:
        loads[0][1].act_func_set_id = nle
        for b, i in loads[1:]:
            b.instructions.remove(i)
    return nc


_NC_CACHE = None


def _get_nc():
    global _NC_CACHE
    if _NC_CACHE is None:
        _NC_CACHE = build()
    return _NC_CACHE


def _prep_inputs(x_local, x_branch, w_qkv, w_proj, gamma, beta):
    bf16 = ml_dtypes.bfloat16
    x_local = np.asarray(x_local, np.float32)
    x_branch = np.asarray(x_branch, np.float32)
    w_qkv = np.asarray(w_qkv, np.float32)
    w_proj = np.asarray(w_proj, np.float32)
    gamma = np.asarray(gamma, np.float32)
    beta = np.asarray(beta, np.float32)

    X = np.concatenate([x_local, x_branch], axis=1).astype(bf16)  # [B, 128, N]
    WT = w_qkv.T.copy()  # [128, 192]
    wq = np.zeros((C2, 128), np.float32)
    wk = np.zeros((C2, 128), np.float32)
    for h in range(H):
        wq[:, 32 * h:32 * h + D] = WT[:, D * h:D * (h + 1)]
        wk[:, 32 * h:32 * h + D] = WT[:, C + D * h:C + D * (h + 1)]
    wv = WT[:, 2 * C:3 * C]
    wp = np.zeros((C2, C), np.float32)
    for h in range(H):
        wp[32 * h:32 * h + D, :] = w_proj[:, D * h:D * (h + 1)].T
    common = dict(
        wq=wq.astype(bf16), wk=wk.astype(bf16), wv=np.ascontiguousarray(wv).astype(bf16),
        wp=wp.astype(bf16),
        gamma=np.ascontiguousarray(gamma.reshape(C, 1)),
        beta=np.ascontiguousarray(beta.reshape(C, 1)),
    )
    return [dict(x=np.ascontiguousarray(X[b]), **common) for b in range(B)]


def kernel(x_local, x_branch, w_qkv, w_proj, gamma, beta, _trace=False, _tmpdir=None):
    nc = _get_nc()
    in_maps = _prep_inputs(x_local, x_branch, w_qkv, w_proj, gamma, beta)
    res = run_bass_kernel_spmd(nc, in_maps, core_ids=list(range(N_CORES)),
                               trace=_trace, tmpdir=_tmpdir)
    out = np.stack([np.asarray(res.results[i]["out"]) for i in range(N_CORES)])
    if _trace:
        kernel._last_results = res
    return out.astype(np.float32)


# revision 16
# speedup vs baseline: 1.1244x; 1.1244x over previous
"""Fused attention block (QKV conv -> 4-head attention -> proj -> BatchNorm -> LeakyReLU)
distributed over 8 trn2 NeuronCores, data-parallel over the batch dim.

Self-contained: hardcodes shapes B=8, C=64, N=2048, H=4.

The kernel is exp-throughput bound (H*N*N = 16.8M exps/core), so the design
keeps both exp-capable engines (ScalarE ACT, VectorE DVE) saturated and hides
everything else under them:
  - scores computed transposed (S^T = K^T Q, keys on partitions) in
    [128 x 512] PSUM tiles through a 5-slot ring; the 4 heads' score matmuls
    issue back-to-back into distinct PE row-groups -> 4x tile concurrency;
  - exp split by head with per-softmax-row purity: heads 0,2 exact exp on
    ScalarE; heads 1,3 on VectorE via a bf16-Schraudolph bit trick
    (int16(FA*s + FB) reinterpreted as bf16 ~= exp(s*scale)); the ~3%
    sawtooth is common to numerator and denominator of those rows and
    largely cancels;
  - PV matmuls col-group tiled (4x concurrent), softmax denominators come
    free from a ones-column folded into the stationary V^T operand;
  - per-query normalization deferred: 1/denom = exp(-ln d) on ScalarE in a
    compact [128,32] layout (DRAM bounce for the transpose), then one
    broadcast-multiply;
  - BatchNorm stats all-reduced across cores ([128,4] f32), folded+replicated
    via a DRAM bounce, inv_std = exp(-0.5*ln(var+eps)); cheap SBUF-only
    elementwise tail ops run on the otherwise idle GpSimd.
"""
import numpy as np
import ml_dtypes

import concourse.bass as bass
import concourse.mybir as mybir
from concourse import bacc, tile
from concourse.bass_utils import run_bass_kernel_spmd

B, C, N, H, D = 8, 64, 2048, 4, 16
C2 = 2 * C           # 128 input channels after concat
NC = 1024            # query-dim chunk
NCH = N // NC        # 2 chunks
MT = N // 128        # 16 key tiles of 128
F32 = mybir.dt.float32
BF16 = mybir.dt.bfloat16
I16 = mybir.dt.int16
SCALE = float(D) ** -0.5
BN_EPS = 1e-5
LEAK = 0.2
N_CORES = 8
CNT = float(B * N)   # batchnorm population count

# bf16-Schraudolph: bf16 bits of exp(s*SCALE) ~= int16(FA*s + FB)
_LN2 = float(np.log(2.0))
FA = 128.0 / _LN2 * SCALE
FB = 127.0 * 128.0 - 4.75

ACT_HEADS = (0, 2)   # exact exp on ScalarE
                     # heads 1,3: fast exp on VectorE

Alu = mybir.AluOpType
Act = mybir.ActivationFunctionType


def build():
    nc = bacc.Bacc("TRN2", target_bir_lowering=False, debug=False,
                   num_devices=N_CORES)
    x_p = nc.declare_dram_parameter("x", [C2, N], BF16, isOutput=False)
    wq_p = nc.declare_dram_parameter("wq", [C2, 128], BF16, isOutput=False)
    wk_p = nc.declare_dram_parameter("wk", [C2, 128], BF16, isOutput=False)
    wv_p = nc.declare_dram_parameter("wv", [C2, C], BF16, isOutput=False)
    wp_p = nc.declare_dram_parameter("wp", [C2, C], BF16, isOutput=False)
    g_p = nc.declare_dram_parameter("gamma", [C, 1], F32, isOutput=False)
    b_p = nc.declare_dram_parameter("beta", [C, 1], F32, isOutput=False)
    out_p = nc.declare_dram_parameter("out", [C, N], F32, isOutput=True)

    with tile.TileContext(nc) as tc:
        with (
            tc.tile_pool(name="sb", bufs=1) as sb,
            tc.tile_pool(name="ps_a", bufs=2, space="PSUM") as ps_a,
            tc.tile_pool(name="ps_d", bufs=2, space="PSUM") as ps_d,
            tc.tile_pool(name="ps_y", bufs=2, space="PSUM") as ps_y,
            tc.tile_pool(name="ps_pv", bufs=1, space="PSUM") as ps_pv,
            tc.tile_pool(name="pp", bufs=4) as pp,
            tc.tile_pool(name="fp", bufs=4) as fp,
            tc.tile_pool(name="ep", bufs=2) as ep,
            tc.tile_pool(name="dram", bufs=2, space="DRAM") as dram,
        ):
            # ---- persistent SBUF tiles
            x_sb = sb.tile([C2, N], BF16, tag="x")
            wq_sb = sb.tile([C2, 128], BF16, tag="wq")
            wk_sb = sb.tile([C2, 128], BF16, tag="wk")
            wv_sb = sb.tile([C2, C], BF16, tag="wv")
            wp_sb = sb.tile([C2, C], BF16, tag="wp")
            g_sb = sb.tile([C2, 1], F32, tag="g")     # gamma replicated x2
            b_sb = sb.tile([C2, 1], F32, tag="b")
            q_sb = sb.tile([C2, N], BF16, tag="q")    # head h rows 32h..32h+16
            k_sb = sb.tile([C2, N], BF16, tag="k")
            # per key-tile, per head: 32 cols = [16 V^T | 1 ones | 15 zeros]
            vt_sb = sb.tile([C2, MT * 128], BF16, tag="vt")
            y_sb = sb.tile([C2, NC], F32, tag="y")    # proj out, fold layout
            stats = sb.tile([C2, 4], F32, tag="stats")  # sum_j0 sum_j1 sq_j0 sq_j1

            # spread prologue loads across engines' DMA queues
            nc.sync.dma_start(x_sb[:, 0:NC], x_p[:, 0:NC])
            nc.gpsimd.dma_start(wq_sb[:], wq_p[:])
            nc.gpsimd.dma_start(wk_sb[:], wk_p[:])
            nc.scalar.dma_start(x_sb[:, NC:N], x_p[:, NC:N])
            nc.sync.dma_start(wv_sb[:], wv_p[:])
            nc.sync.dma_start(wp_sb[:], wp_p[:])
            nc.gpsimd.dma_start(g_sb[0:C, :], g_p[:])
            nc.gpsimd.dma_start(g_sb[C:C2, :], g_p[:])
            nc.scalar.dma_start(b_sb[0:C, :], b_p[:])
            nc.scalar.dma_start(b_sb[C:C2, :], b_p[:])

            # ---- QKV projections (heads padded to 32-partition groups).
            # Evacuate per 512-col piece, alternating DVE/ACT so the first
            # score matmul isn't gated on one engine.
            def qk_chunk(dst, w, c, eng_flip):
                for j in range(NC // 512):
                    qp = ps_a.tile([C2, 512], F32, tag="sa", name="qp")
                    nc.tensor.matmul(
                        qp[:], lhsT=w[:],
                        rhs=x_sb[:, NC * c + 512 * j: NC * c + 512 * (j + 1)])
                    dslice = dst[:, NC * c + 512 * j: NC * c + 512 * (j + 1)]
                    if (j + eng_flip) % 2 == 0:
                        nc.vector.tensor_copy(dslice, qp[:])
                    else:
                        nc.scalar.copy(dslice, qp[:])

            qk_chunk(q_sb, wq_sb, 0, 0)
            qk_chunk(k_sb, wk_sb, 0, 1)
            qk_chunk(q_sb, wq_sb, 1, 0)
            qk_chunk(k_sb, wk_sb, 1, 1)

            # V^T zero fill + ones columns on the otherwise-idle gpsimd
            nc.gpsimd.memset(vt_sb[:], 0.0)
            ones_ap = vt_sb[:].rearrange(
                "q (p h e) -> q p h e", p=MT, h=H, e=32)[:, :, :, 16:17]
            nc.gpsimd.memset(ones_ap, 1.0)

            # V^T: 16 key tiles, two 512-col psum ring slots of 8 tiles each
            for half in range(2):
                vp = ps_a.tile([C2, 512], F32, tag="sa", name="vp")
                for p in range(8):
                    pk = 8 * half + p
                    nc.tensor.matmul(vp[:, C * p:C * (p + 1)],
                                     lhsT=x_sb[:, 128 * pk:128 * (pk + 1)],
                                     rhs=wv_sb[:])
                vt_dst = vt_sb[:, 1024 * half:1024 * (half + 1)].rearrange(
                    "q (p h e) -> q p h e", p=8, h=H, e=32)[:, :, :, 0:16]
                vt_src = vp[:].rearrange(
                    "q (p h d) -> q p h d", p=8, h=H, d=D)
                nc.vector.tensor_copy(vt_dst, vt_src)

            def epilogue_head(c, pv):
                """Evacuate pv to SBUF at the chunk boundary (frees the psum
                slot); split across ACT/DVE so neither queue stalls long."""
                pvs = ep.tile([C2, NC], F32, tag="pvs")
                nc.scalar.copy(pvs[:, 0:512], pv[:, 0:512])
                nc.vector.tensor_copy(pvs[:, 512:NC], pv[:, 512:NC])
                return pvs

            def epilogue_tail(c, pvs):
                """Normalize chunk-c attention output, project, evac + stats.
                Emitted a few steps into the NEXT chunk so the exp engines
                keep streaming; latency lives on DMA/GpSimd/PE instead."""
                # denominators pvs[32h+16, :] -> DRAM [4, NC] -> compact
                # [128, 32]; reciprocal on DVE (tiny); back out + bcast
                den_d = dram.tile([H, NC], F32, tag="den_d")
                for h in range(H):
                    nc.sync.dma_start(den_d[h:h + 1, :],
                                      pvs[32 * h + 16:32 * h + 17, :])
                den_c = ep.tile([C2, NC // 32], F32, tag="den_c")
                nc.sync.dma_start(
                    den_c[:],
                    den_d[:].rearrange("h (p q) -> (h p) q", p=32, q=32))
                rcp = ep.tile([C2, NC // 32], F32, tag="rcp")
                nc.vector.reciprocal(rcp[:], den_c[:])
                rec_d = dram.tile([H, NC], F32, tag="rec_d")
                nc.sync.dma_start(
                    rec_d[:].rearrange("h (p q) -> (h p) q", p=32, q=32),
                    rcp[:])
                rbc = ep.tile([C2, NC], F32, tag="rbc")
                for h in range(H):
                    nc.sync.dma_start(
                        rbc[32 * h:32 * h + 32, :],
                        rec_d[h:h + 1, :].partition_broadcast(32))
                on = ep.tile([C2, NC], BF16, tag="on")
                nc.gpsimd.tensor_tensor(on[:], pvs[:], rbc[:], op=Alu.mult)
                # projection into s-ring slots; chunk c rows 64c..64c+64
                r = slice(64 * c, 64 * c + 64)
                for j in range(NC // 512):
                    yp = ps_y.tile([C2, 512], F32, tag="sy", name="yp")
                    nc.tensor.matmul(yp[r, :],
                                     lhsT=wp_sb[:], rhs=on[:, 512 * j:512 * (j + 1)],
                                     tile_position=(0, 64 * c))
                    nc.vector.tensor_scalar(y_sb[r, 512 * j:512 * (j + 1)],
                                            yp[r, :], 1.0, 0.0,
                                            op0=Alu.mult, op1=Alu.add,
                                            accum_out=stats[r, j:j + 1])
                    ysq = ep.tile([C2, 512], F32, tag="ysq")
                    nc.vector.scalar_tensor_tensor(
                        ysq[r, :], y_sb[r, 512 * j:512 * (j + 1)], 0.0,
                        y_sb[r, 512 * j:512 * (j + 1)], op0=Alu.add,
                        op1=Alu.mult, accum_out=stats[r, 2 + j:3 + j])

            # ---- attention: per (chunk, key-tile, j-half): 4 S matmuls into
            # the 5-deep psum ring (distinct PE row-groups -> concurrent),
            # exp on ScalarE (heads 0,2) / VectorE (heads 1,3 bit-trick),
            # then 4 col-tiled PV matmuls accumulate into the pv slot.  The
            # PV group lags one (t,j) step so the PE emits the NEXT step's
            # score matmuls before the PV that waits on this step's exps —
            # the exp engines then never wait on a fresh score tile.
            pend_tail = None
            for c in range(NCH):
                pv = ps_pv.tile([C2, NC], F32, tag="pv", name="pv")
                pend = []

                def flush_pv(pv=pv):
                    for pt, t, j, h in pend.pop(0):
                        nc.tensor.matmul(
                            pv[32 * h:32 * h + 32, 512 * j:512 * (j + 1)],
                            lhsT=vt_sb[:, 128 * t + 32 * h: 128 * t + 32 * h + 32],
                            rhs=pt[:],
                            start=(t == 0), stop=(t == MT - 1),
                            skip_group_check=True,
                            tile_position=(0, 32 * h))

                for t in range(MT):
                    for j in range(NC // 512):
                        if c == 1 and t == 2 and j == 0 and pend_tail is not None:
                            epilogue_tail(*pend_tail)
                            pend_tail = None
                        for pr in range(2):
                            sps = []
                            for h in (2 * pr, 2 * pr + 1):
                                if h in ACT_HEADS:
                                    sp = ps_a.tile([C2, 512], F32, tag="sa",
                                                   name="spa")
                                else:
                                    sp = ps_d.tile([C2, 512], F32, tag="sd",
                                                   name="spd")
                                nc.tensor.matmul(
                                    sp[:],
                                    lhsT=k_sb[32 * h:32 * h + 16, 128 * t:128 * (t + 1)],
                                    rhs=q_sb[32 * h:32 * h + 16,
                                             NC * c + 512 * j: NC * c + 512 * (j + 1)],
                                    tile_position=(32 * h, 0))
                                sps.append((sp, h))
                            pts = []
                            for sp, h in sps:
                                if h in ACT_HEADS:
                                    pt = pp.tile([C2, 512], BF16, tag="p")
                                    nc.scalar.activation(pt[:], sp[:], Act.Exp,
                                                         scale=SCALE)
                                    pts.append((pt[:], t, j, h))
                                else:
                                    pti = fp.tile([C2, 512], I16, tag="pf")
                                    nc.vector.tensor_scalar(pti[:], sp[:], FA,
                                                            FB, op0=Alu.mult,
                                                            op1=Alu.add)
                                    pts.append((pti[:].bitcast(BF16), t, j, h))
                            pend.append(pts)
                            # 2-sub-step lag keeps the next scores dispatched
                            # ahead of the exp-gated PV pair
                            if len(pend) > 2:
                                flush_pv()
                while pend:
                    flush_pv()
                pend_tail = (c, epilogue_head(c, pv))
            epilogue_tail(*pend_tail)

            # ---- cross-core reduce of batchnorm stats (prefold j-pairs
            # to [128, 2] so the collective moves 1KB)
            st2 = sb.tile([C2, 2], F32, tag="st2")
            nc.gpsimd.tensor_add(st2[:], stats[:, 0:4:2], stats[:, 1:4:2])
            st_in = dram.tile([C2, 2], F32, tag="st_in")
            st_out = dram.tile([N_CORES * C2, 2], F32, tag="st_out")
            nc.gpsimd.dma_start(st_in[:], st2[:])
            nc.gpsimd.collective_compute(
                "AllGather", Alu.bypass,
                replica_groups=[list(range(N_CORES))],
                ins=[st_in.opt()], outs=[st_out.opt()])
            # load all 8 ranks' [C2,2] as [C2, 16], reduce ranks, then fold
            # the two channel halves (replicated to all 128 partitions)
            ag = sb.tile([C2, 16], F32, tag="ag")
            nc.sync.dma_start(
                ag[:].rearrange("p (r j) -> p r j", r=N_CORES, j=2),
                st_out[:].rearrange("(r p) j -> p r j", r=N_CORES, p=C2))
            r8 = sb.tile([C2, 8], F32, tag="r8")
            nc.vector.tensor_add(r8[:], ag[:, 0:8], ag[:, 8:16])
            r4 = sb.tile([C2, 4], F32, tag="r4")
            nc.vector.tensor_add(r4[:], r8[:, 0:4], r8[:, 4:8])
            rsum = sb.tile([C2, 2], F32, tag="rsum")
            nc.vector.tensor_add(rsum[:], r4[:, 0:2], r4[:, 2:4])
            rs_d = dram.tile([C2, 2], F32, tag="rs_d")
            nc.sync.dma_start(rs_d[:], rsum[:])
            fa = sb.tile([C2, 2], F32, tag="fa")
            fb = sb.tile([C2, 2], F32, tag="fb")
            nc.sync.dma_start(fa[:], rs_d[:])
            nc.sync.dma_start(fb[0:C, :], rs_d[C:C2, :])
            nc.sync.dma_start(fb[C:C2, :], rs_d[0:C, :])
            # ---- finalize, single-engine chain on vector (ln/exp on ScalarE)
            me = sb.tile([C2, 2], F32, tag="me")     # [mean | E x^2]
            nc.vector.tensor_add(me[:], fa[:], fb[:])
            nc.vector.tensor_scalar_mul(me[:], me[:], 1.0 / CNT)
            mean = me[:, 0:1]
            msq = sb.tile([C2, 1], F32, tag="msq")
            nc.vector.tensor_mul(msq[:], me[:, 0:1], me[:, 0:1])
            var = sb.tile([C2, 1], F32, tag="var")
            nc.vector.tensor_sub(var[:], me[:, 1:2], msq[:])
            eps_t = sb.tile([C2, 1], F32, tag="eps")
            nc.gpsimd.memset(eps_t[:], BN_EPS)
            lnv = sb.tile([C2, 1], F32, tag="lnv")
            nc.scalar.activation(lnv[:], var[:], Act.Ln, bias=eps_t[:, 0:1])
            istd = sb.tile([C2, 1], F32, tag="istd")
            nc.scalar.activation(istd[:], lnv[:], Act.Exp, scale=-0.5)
            sc = sb.tile([C2, 1], F32, tag="sc")
            nc.vector.tensor_mul(sc[:], g_sb[:], istd[:])
            msc = sb.tile([C2, 1], F32, tag="msc")
            nc.vector.tensor_scalar(msc[:], mean, sc[:, 0:1], None, op0=Alu.mult)
            sh = sb.tile([C2, 1], F32, tag="sh")
            nc.vector.tensor_sub(sh[:], b_sb[:], msc[:])

            # ---- normalize + LeakyReLU + store (gpsimd + vector split)
            yn = ep.tile([C2, NC], F32, tag="rbc")
            nc.vector.tensor_scalar(yn[:], y_sb[:],
                                    sc[:, 0:1], sh[:, 0:1],
                                    op0=Alu.mult, op1=Alu.add)
            yl = ep.tile([C2, NC], F32, tag="on2")
            nc.vector.scalar_tensor_tensor(yl[:], yn[:], LEAK, yn[:],
                                           op0=Alu.mult, op1=Alu.max)

            nc.sync.dma_start(out_p[:, 0:NC], yl[0:C, :])
            nc.sync.dma_start(out_p[:, NC:N], yl[C:C2, :])

    nc.compile()

    # Post-compile surgery: one activation table set covers Exp+Ln; point the
    # first inserted load at it and drop the rest (loads are inserted after
    # semaphore generation, so they carry no sync state and removal is safe).
    from concourse.hw_specs import get_activation_tables
    tabs = list(get_activation_tables(nc.m.arch).keys())
    nle = tabs.index("natural_log_exp_and_others")
    loads = [(b, i) for b in nc.main_func.blocks for i in b.instructions
             if isinstance(i, mybir.InstLoadActFuncSet)]
    if loads:
        loads[0][1].act_func_set_id = nle
        for b, i in loads[1:]:
            b.instructions.remove(i)
    return nc


_NC_CACHE = None


def _get_nc():
    global _NC_CACHE
    if _NC_CACHE is None:
        _NC_CACHE = build()
    return _NC_CACHE


def _prep_inputs(x_local, x_branch, w_qkv, w_proj, gamma, beta):
    bf16 = ml_dtypes.bfloat16
    x_local = np.asarray(x_local, np.float32)
    x_branch = np.asarray(x_branch, np.float32)
    w_qkv = np.asarray(w_qkv, np.float32)
    w_proj = np.asarray(w_proj, np.float32)
    gamma = np.asarray(gamma, np.float32)
    beta = np.asarray(beta, np.float32)

    X = np.concatenate([x_local, x_branch], axis=1).astype(bf16)  # [B, 128, N]
    WT = w_qkv.T.copy()  # [128, 192]
    wq = np.zeros((C2, 128), np.float32)
    wk = np.zeros((C2, 128), np.float32)
    for h in range(H):
        wq[:, 32 * h:32 * h + D] = WT[:, D * h:D * (h + 1)]
        wk[:, 32 * h:32 * h + D] = WT[:, C + D * h:C + D * (h + 1)]
    wv = WT[:, 2 * C:3 * C]
    wp = np.zeros((C2, C), np.float32)
    for h in range(H):
        wp[32 * h:32 * h + D, :] = w_proj[:, D * h:D * (h + 1)].T
    common = dict(
        wq=wq.astype(bf16), wk=wk.astype(bf16), wv=np.ascontiguousarray(wv).astype(bf16),
        wp=wp.astype(bf16),
        gamma=np.ascontiguousarray(gamma.reshape(C, 1)),
        beta=np.ascontiguousarray(beta.reshape(C, 1)),
    )
    return [dict(x=np.ascontiguousarray(X[b]), **common) for b in range(B)]


def kernel(x_local, x_branch, w_qkv, w_proj, gamma, beta, _trace=False, _tmpdir=None):
    nc = _get_nc()
    in_maps = _prep_inputs(x_local, x_branch, w_qkv, w_proj, gamma, beta)
    res = run_bass_kernel_spmd(nc, in_maps, core_ids=list(range(N_CORES)),
                               trace=_trace, tmpdir=_tmpdir)
    out = np.stack([np.asarray(res.results[i]["out"]) for i in range(N_CORES)])
    if _trace:
        kernel._last_results = res
    return out.astype(np.float32)
